# revision 5
# baseline (speedup 1.0000x reference)
"""Trainium2 Bass kernel for nn_DenseInputEncoder (to_dense_adj-style scatter).

Strategy (data-parallel over graphs, 8 graphs per NeuronCore):
  The output dense_pair_h[b, h, r, c] is a mostly-zero dense tensor built from
  ~2k scattered (r, c) cells per graph.  Instead of DMA scatter, each 512-cell
  output "window" is produced DENSE by a TensorE matmul:
      out[h, cell] = sum_items V[item, h] * onehot[item, cell]
  where onehot[item, cell] = (rc_local[item] == iota[cell]) is built on the
  vector engines.  The matmul simultaneously performs the scatter, sums
  duplicate cells, and fills untouched cells with exact zeros.  Item values
  V = feat96 @ W96 unify the edge/pair/node-diagonal encoders (features are
  placed in disjoint 96-dim blocks on the host, so one weight matrix serves
  all three).  Graph pairs share a [128 x 8192] SBUF slab (partitions 0-63 =
  graph a's h-planes, 64-127 = graph b's) which DMAs out as large contiguous
  writes; structurally-empty windows are written from a static zero tile.
"""

import numpy as np
import ml_dtypes
from contextlib import ExitStack

import concourse.mybir as mybir
import concourse.tile as tile
from concourse import bacc
from concourse.bass_utils import run_bass_kernel_spmd

B = 64          # graphs
N = 128         # max nodes per graph (dense padding)
H = 64          # hidden dim
NCORES = 8
GPC = B // NCORES  # graphs per core
WIN = 512       # cells per window (one PSUM bank at fp32)
NWIN = (N * N) // WIN  # 32 windows per graph
P = 128         # partitions / matmul contraction size
F = 96          # unified feature dim: [edge 32 | pair 16 | node 32 | loop 16]

_f32 = mybir.dt.float32
_bf16 = mybir.dt.bfloat16

_program_cache = {}


def _host_prep(inputs):
    """Index math + feature packing on host (numpy).  Returns per-core input
    arrays, the uniform chunk plan, and the (host-computed) node mask."""
    batch = np.asarray(inputs["batch"]).astype(np.int64)
    edge_index = np.asarray(inputs["edge_index"]).astype(np.int64)
    pair_index = np.asarray(inputs["pair_index"]).astype(np.int64)
    node_x = np.asarray(inputs["node_x"], dtype=np.float32)
    loop_x = np.asarray(inputs["loop_x"], dtype=np.float32)
    edge_attr = np.asarray(inputs["edge_attr"], dtype=np.float32)
    pair_x = np.asarray(inputs["pair_x"], dtype=np.float32)

    NT = batch.shape[0]
    E = edge_index.shape[1]

    # position of each node within its graph (to_dense_batch semantics)
    counts = np.bincount(batch, minlength=B)
    starts = np.concatenate([[0], np.cumsum(counts)[:-1]])
    pos = np.arange(NT, dtype=np.int64) - starts[batch]

    # unified item list: edges, pairs, node-diagonal entries
    e0, e1 = edge_index
    p0, p1 = pair_index
    b_it = np.concatenate([batch[e0], batch[p0], batch])
    r_it = np.concatenate([pos[e0], pos[p0], pos])
    c_it = np.concatenate([pos[e1], pos[p1], pos])
    n_items = b_it.shape[0]

    feat = np.zeros((n_items, F), np.float32)
    feat[:E, 0:32] = edge_attr
    feat[E : 2 * E, 32:48] = pair_x
    feat[2 * E :, 48:80] = node_x
    feat[2 * E :, 80:96] = loop_x

    # out-of-bounds scatter indices are dropped (jax .at[] default)
    valid = (r_it >= 0) & (r_it < N) & (c_it >= 0) & (c_it < N) & (b_it >= 0) & (b_it < B)
    b_v, r_v, c_v = b_it[valid], r_it[valid], c_it[valid]
    feat_v = feat[valid]
    cell = r_v * N + c_v
    w_v = cell // WIN
    rc_local = (cell % WIN).astype(np.float32)
    core_v = b_v // GPC
    g_v = b_v % GPC

    # counts per (core, g, w) -> uniform chunk plan (max over cores)
    key = (core_v * GPC + g_v) * NWIN + w_v
    cnt = np.bincount(key, minlength=NCORES * GPC * NWIN).reshape(NCORES, GPC, NWIN)
    C_gw = -(-cnt.max(axis=0) // P)  # [GPC, NWIN] chunks needed (0 = dead window)

    # chunk table: contiguous chunk ids grouped by g, then w
    t_start = np.zeros((GPC, NWIN), np.int64)
    plan = []  # per g: list of (w, n_chunks, t0)
    T = 0
    for g in range(GPC):
        gplan = []
        for w in range(NWIN):
            c_ = int(C_gw[g, w])
            if c_ == 0:
                continue
            t_start[g, w] = T
            gplan.append((w, c_, T))
            T += c_
        plan.append(gplan)

    # slot assignment: rank of each item within its (core, g, w) group
    order = np.argsort(key, kind="stable")
    key_s = key[order]
    grp_first = np.concatenate([[0], np.cumsum(np.bincount(key_s))[:-1]])
    j = np.arange(key_s.shape[0]) - grp_first[key_s]

    g_s = g_v[order]
    w_s = w_v[order]
    col = t_start[g_s, w_s] * P + j  # column within the core's feats array
    core_s = core_v[order]
    rc_s = rc_local[order]
    feat_s = feat_v[order]

    feats_cores = []
    rc_cores = []
    for k in range(NCORES):
        m = core_s == k
        fa = np.zeros((F, T * P), np.float32)
        fa[:, col[m]] = feat_s[m].T
        ra = np.full((P, T), -1.0, np.float32)
        ra[col[m] % P, col[m] // P] = rc_s[m]
        feats_cores.append(fa.astype(ml_dtypes.bfloat16))
        rc_cores.append(ra)

    W96 = np.concatenate(
        [
            np.asarray(inputs["W_edge"], np.float32),
            np.asarray(inputs["W_pair"], np.float32),
            np.asarray(inputs["W_node"], np.float32),
            np.asarray(inputs["W_loop"], np.float32),
        ],
        axis=0,
    ).astype(ml_dtypes.bfloat16)

    mask = np.zeros((B, N), bool)
    nv = (pos >= 0) & (pos < N) & (batch >= 0) & (batch < B)
    mask[batch[nv], pos[nv]] = True

    # which slab column-range is ever written by live windows, per g-pair
    live_w = sorted({w for g in range(GPC) for (w, _, _) in plan[g]})
    return feats_cores, rc_cores, W96, plan, T, live_w, mask


def _build_program(plan_key, plan, T, live_w):
    """Build + compile the (SPMD-uniform) Bass program."""
    nc = bacc.Bacc("TRN2", num_devices=NCORES)

    feats_d = nc.dram_tensor("feats", [F, T * P], _bf16, kind="ExternalInput")
    rc_d = nc.dram_tensor("rc", [P, T], _f32, kind="ExternalInput")
    w96_d = nc.dram_tensor("w96", [F, H], _bf16, kind="ExternalInput")
    out_d = nc.dram_tensor("out", [GPC, H, N, N], _f32, kind="ExternalOutput")
    out_v = out_d.ap().rearrange("g h r c -> (g h) (r c)")

    # live window columns form the "live" slab region; everything to the right
    # of the highest live window is written from a static zero tile.
    w_hi = (max(live_w) + 1) if live_w else 0
    live_cols = w_hi * WIN
    zero_cols = N * N - live_cols

    with tile.TileContext(nc) as tc, ExitStack() as ctx:
        const = ctx.enter_context(tc.tile_pool(name="const", bufs=1))
        feats_p = ctx.enter_context(tc.tile_pool(name="feats", bufs=4))
        v_p = ctx.enter_context(tc.tile_pool(name="v", bufs=4))
        oh_p = ctx.enter_context(tc.tile_pool(name="oh", bufs=6))
        slab_p = ctx.enter_context(tc.tile_pool(name="slab", bufs=2))
        pv_p = ctx.enter_context(tc.tile_pool(name="pv", bufs=3, space="PSUM"))
        pw_p = ctx.enter_context(tc.tile_pool(name="pw", bufs=4, space="PSUM"))

        iota_t = const.tile([P, WIN], dtype=_f32)
        nc.gpsimd.iota(
            iota_t[:], pattern=[[1, WIN]], base=0, channel_multiplier=0,
            allow_small_or_imprecise_dtypes=True,
        )
        w96_t = const.tile([F, H], dtype=_bf16)
        nc.sync.dma_start(out=w96_t[:], in_=w96_d.ap())
        rc_t = const.tile([P, T], dtype=_f32)
        nc.sync.dma_start(out=rc_t[:], in_=rc_d.ap())

        if zero_cols:
            zero_t = const.tile([P, zero_cols], dtype=_f32)
            nc.vector.memset(zero_t[:], 0.0)

        oh_i = 0
        cp_i = 0
        for pair in range(GPC // 2):
            slab = slab_p.tile([P, live_cols], dtype=_f32)
            v_ts = []
            for gg in range(2):
                g = 2 * pair + gg
                gplan = plan[g]
                nch = sum(c_ for (_, c_, _) in gplan)
                t0g = gplan[0][2] if gplan else 0
                feats_t = feats_p.tile([F, max(nch, 1) * P], dtype=_bf16, tag="feats")
                v_t = v_p.tile([P, max(nch, 1) * H], dtype=_bf16, tag=f"v{gg}")
                if nch:
                    nc.sync.dma_start(
                        out=feats_t[:], in_=feats_d.ap()[:, t0g * P : (t0g + nch) * P]
                    )
                    for tl in range(nch):  # local chunk index
                        pv = pv_p.tile([P, H], dtype=_f32)
                        nc.tensor.matmul(
                            out=pv[:],
                            lhsT=feats_t[:, tl * P : (tl + 1) * P],
                            rhs=w96_t[:],
                            start=True,
                            stop=True,
                        )
                        nc.vector.tensor_copy(
                            out=v_t[:, tl * H : (tl + 1) * H], in_=pv[:]
                        )
                v_ts.append((v_t, t0g))

            for w in live_w:
                ps = pw_p.tile([P, WIN], dtype=_f32)
                for gg in range(2):
                    g = 2 * pair + gg
                    v_t, t0g = v_ts[gg]
                    ent = [e for e in plan[g] if e[0] == w]
                    if not ent:
                        # graph has no live chunks for this window: zero the
                        # half (vector engine only — GpSimd can't touch PSUM)
                        nc.vector.memset(ps[gg * H : (gg + 1) * H, :], 0.0)
                        continue
                    (_, c_, t0) = ent[0]
                    for cc in range(c_):
                        t = t0 + cc  # global chunk id
                        tl = t - t0g
                        oh = oh_p.tile([P, WIN], dtype=_bf16, tag="oh")
                        eng = nc.vector
                        eng.tensor_tensor(
                            out=oh[:],
                            in0=iota_t[:],
                            in1=rc_t[:, t : t + 1].to_broadcast([P, WIN]),
                            op=mybir.AluOpType.is_equal,
                        )
                        oh_i += 1
                        nc.tensor.matmul(
                            out=ps[gg * H : (gg + 1) * H, :],
                            lhsT=v_t[:, tl * H : (tl + 1) * H],
                            rhs=oh[:],
                            start=(cc == 0),
                            stop=(cc == c_ - 1),
                        )
                eng = nc.vector if (cp_i % 2 == 0) else nc.scalar
                if hasattr(eng, "tensor_copy"):
                    eng.tensor_copy(out=slab[:, w * WIN : (w + 1) * WIN], in_=ps[:])
                else:
                    eng.copy(out=slab[:, w * WIN : (w + 1) * WIN], in_=ps[:])
                cp_i += 1

            # dead windows inside the live range (none for typical data, but
            # keep general): write zeros into slab columns not in live_w
            dead = [w for w in range(w_hi) if w not in live_w]
            for w in dead:
                nc.gpsimd.memset(slab[:, w * WIN : (w + 1) * WIN], 0.0)

            rows = slice(pair * P, (pair + 1) * P)
            nc.sync.dma_start(out=out_v[rows, 0:live_cols], in_=slab[:])
            if zero_cols:
                nc.sync.dma_start(out=out_v[rows, live_cols:], in_=zero_t[:])

    nc.compile()
    return nc


def _prepare(inputs):
    """Host prep + (cached) program build.  Returns (nc, in_maps, mask)."""
    feats_cores, rc_cores, W96, plan, T, live_w, mask = _host_prep(inputs)

    plan_key = (T, tuple(tuple(g) for g in plan), tuple(live_w))
    nc = _program_cache.get(plan_key)
    if nc is None:
        nc = _build_program(plan_key, plan, T, live_w)
        _program_cache[plan_key] = nc

    in_maps = [
        {"feats": feats_cores[k], "rc": rc_cores[k], "w96": W96}
        for k in range(NCORES)
    ]
    return nc, in_maps, mask


def kernel(**inputs):
    nc, in_maps, mask = _prepare(inputs)
    res = run_bass_kernel_spmd(nc, in_maps, core_ids=list(range(NCORES)))
    global _last_results
    _last_results = res
    dense = np.concatenate([r["out"] for r in res.results], axis=0)
    return dense, mask


_last_results = None
